# revision 1
# baseline (speedup 1.0000x reference)
"""Trainium2 Bass kernel for nn_AttentionMono (B=8, K=Q=T=256, A=64).

Sharding: data-parallel over batch B across the 8 NeuronCores (B == n_cores,
zero communication).  Each core computes one batch:

    key   = Wk  @ key_tokens[b].T            # [A, K]
    query = Wq  @ query_tokens[b].T          # [A, Q]
    value = Wvd @ key_tokens[b].T            # [A, K]
    x[k,c,q]   = sum_a Wa[c,a]*key[a,k]*query[a,q]        (logits)
    swishmax over q (axis=-2 of [B,K,Q,C]) with NOT_EPSILON=1:
        m = max_q x ; n0 = x*exp(x-m) ; scale = n0/(sum_q|n0| + 1)
    out[q,t] = sum_a Wvu[t,a] * sum_k value[a,k]*scale[k,c=a,q]

Reformulation used on device (exactly equal in fp32, verified to ~6e-7):
    n = x*exp(x) ; D = sum_q |n| + max_q exp(x) ; scale = n/D
which removes the per-(k,c) bias from the exp (logits are O(2), no overflow).

Per-core dataflow (K=256 processed as 128 pairs of 2 k's):
  - logits for a pair: one PE matmul  out[128p=(2k x 64c), 256q] =
        lhsT_bd.T @ qT2   with lhsT_bd[128,128] = blockdiag(WaT,WaT) scaled
        per-partition by key2[:,j] (one tensor_scalar build).
  - ACT: e = Exp(logits) grouped over 4 pairs; DVE: max_q e (grouped reduce);
    DVE: n = logits*e ; ACT: |n| with accum_out -> sum_q|n| per pair.
  - K-sum on PE: vsum[64c,256q] += W_j.T @ n_j accumulated in PSUM over all
    128 pairs, where W_j[128,64] = SEL2 * (value2[:,j]/D[:,j]) per-partition.
  - epilogue: out[q,t] = vsum.T @ WvuT via 2 PE matmuls.
"""

import os
import sys

import numpy as np

for _p in ("/root/.axon_site", "/root/.axon_site/_ro/trn_rl_repo",
           "/root/.axon_site/_ro/pypackages", "/opt/trn_rl_repo"):
    if os.path.isdir(_p) and _p not in sys.path:
        sys.path.append(_p)

B, K, Q, T, A = 8, 256, 256, 256, 64
PAIRS = K // 2          # 128
G = 4                   # pairs per elementwise group ([128, 1024] tiles)
NG = PAIRS // G         # 32
SGG = 2                 # groups per stats supergroup
SGP = SGG * G           # pairs per supergroup (8)

# 'gpsimd' or 'vector': engine for the lhsT/W tensor_scalar builds
BUILD_ENGINE = os.environ.get("AM_BUILD_ENGINE", "gpsimd")
# matmul dtype: 'f32' (exact, 4 cyc/row) or 'f32r' (replicated, 1 cyc/row)
MM_DTYPE = os.environ.get("AM_MM_DTYPE", "f32r")
# per-pair engine cycle for the |n| row-sum: subset of "act,dve"
# (walrus rejects tensor_scalar with accum_out on the Pool engine)
ABS_SPLIT = os.environ.get("AM_ABS_SPLIT", "dve,act,dve,act,dve,act,dve,act,dve,dve,act,dve,act,dve,act,dve")
# engine/style for max_q(e): 'dve_ts', 'gps_ts', or 'dve_reduce'
MAXE_MODE = os.environ.get("AM_MAXE", "dve_reduce")
# how many supergroups of lag between logits production and SEL consumption
SEL_LAG = int(os.environ.get("AM_SEL_LAG", "1"))
# groups of lhsT-build prefetch
PF = int(os.environ.get("AM_PF", "4"))
# 'gps' or 'dve': engine for the per-supergroup stats add/mul
STATS_ENGINE = os.environ.get("AM_STATS", "gps")

_nc_cache = {}


def build_program(build_engine=BUILD_ENGINE, mm_dtype=MM_DTYPE,
                  abs_split=ABS_SPLIT, sel_lag=SEL_LAG, maxe_mode=MAXE_MODE,
                  pf=PF, stats_engine=STATS_ENGINE):
    import concourse.bass as bass
    import concourse.bacc as bacc
    import concourse.mybir as mybir
    from concourse import tile

    f32 = mybir.dt.float32
    # fr: dtype of tensors feeding the TensorEngine. float32r streams at
    # 1 cyc/row (vs 4 for fp32) but requires producers to write rounded
    # values, so the feeding tiles are typed float32r and non-matmul
    # readers bitcast back to f32.
    fr = mybir.dt.float32r if mm_dtype == "f32r" else f32
    AF = mybir.ActivationFunctionType
    AX = mybir.AxisListType
    OP = mybir.AluOpType

    def mm_ap(ap):
        return ap

    def as_f32(ap):
        return ap.bitcast(f32) if fr != f32 else ap

    nc = bacc.Bacc("TRN2", target_bir_lowering=False, debug=False)

    kt = nc.dram_tensor("kt", [K, T], f32, kind="ExternalInput")
    qt = nc.dram_tensor("qt", [Q, T], f32, kind="ExternalInput")
    wkt = nc.dram_tensor("wkt", [128, 128], f32, kind="ExternalInput")
    wqt = nc.dram_tensor("wqt", [128, 128], f32, kind="ExternalInput")
    wvdt = nc.dram_tensor("wvdt", [128, 128], f32, kind="ExternalInput")
    if mm_dtype == "f32r":
        # f32r matmuls require dst partition base 0: use one [128,128]
        # block-diagonal lhsT per pair instead of even/odd 64x64 tiles
        wabd = nc.dram_tensor("wabd", [128, 128], f32, kind="ExternalInput")
    else:
        wat2 = nc.dram_tensor("wat2", [128, A], f32, kind="ExternalInput")
    wvut = nc.dram_tensor("wvut", [A, T], f32, kind="ExternalInput")
    sel2 = nc.dram_tensor("sel2", [128, A], f32, kind="ExternalInput")
    wlgx = nc.dram_tensor("wlgx", [128, SGP * 128], f32, kind="ExternalInput")
    sel2x = nc.dram_tensor("sel2x", [128, SGP * A], f32, kind="ExternalInput")
    ident = nc.dram_tensor("ident", [128, 128], f32, kind="ExternalInput")
    out = nc.dram_tensor("out", [Q, T], f32, kind="ExternalOutput")

    with tile.TileContext(nc) as tc:
        build_eng = nc.gpsimd if build_engine == "gpsimd" else nc.vector
        with (
            tc.tile_pool(name="const", bufs=1) as cpool,
            tc.tile_pool(name="persist", bufs=1) as ppool,
            tc.tile_pool(name="vsum_ps", bufs=1, space="PSUM") as vsum_pool,
        ):
            # ---- token loads first (critical path), weights behind ----
            kt_sb = []
            qt_sb = []
            for c in range(2):
                t1 = ppool.tile([128, T], f32, name=f"kt_sb{c}")
                nc.sync.dma_start(t1[:], kt[c * 128:(c + 1) * 128, :])
                kt_sb.append(t1)
                t2 = ppool.tile([128, T], f32, name=f"qt_sb{c}")
                nc.scalar.dma_start(t2[:], qt[c * 128:(c + 1) * 128, :])
                qt_sb.append(t2)
            ident_sb = cpool.tile([128, 128], f32, name="ident_sb")
            nc.sync.dma_start(ident_sb[:], ident[:])
            wkt_sb = cpool.tile([128, 128], f32, name="wkt_sb")
            nc.scalar.dma_start(wkt_sb[:], wkt[:])
            wqt_sb = cpool.tile([128, 128], f32, name="wqt_sb")
            nc.sync.dma_start(wqt_sb[:], wqt[:])
            if mm_dtype == "f32r":
                lhsT_cols = 128
            else:
                lhsT_cols = A
            wlgx_sb = cpool.tile([128, SGP * lhsT_cols], f32, name="wlgx_sb")
            nc.scalar.dma_start(wlgx_sb[:], wlgx[:, :SGP * lhsT_cols])
            # late weights (not needed until the first stage_sel / epilogue)
            sel2x_sb = cpool.tile([128, SGP * A], f32, name="sel2x_sb")
            nc.sync.dma_start(sel2x_sb[:], sel2x[:])
            wvdt_sb = cpool.tile([128, 128], f32, name="wvdt_sb")
            nc.scalar.dma_start(wvdt_sb[:], wvdt[:])
            wvut_sb = cpool.tile([A, T], f32, name="wvut_sb")
            nc.sync.dma_start(wvut_sb[:], wvut[:])
            wvut_r = cpool.tile([A, T], fr, name="wvut_r")
            nc.vector.tensor_copy(wvut_r[:], wvut_sb[:])

            # persistent SBUF targets
            ktT_sb = [ppool.tile([128, K], f32, name=f"ktT_sb{i}")
                      for i in range(2)]  # [t-chunk][t, k]
            qtT_sb = [ppool.tile([128, Q], f32, name=f"qtT_sb{i}")
                      for i in range(2)]
            key2_sb = ppool.tile([128, PAIRS], f32, name="key2_sb")
            val2_sb = ppool.tile([128, PAIRS], f32, name="val2_sb")
            qT2_sb = ppool.tile([128, Q], fr, name="qT2_sb")
            stats_maxe = ppool.tile([128, PAIRS], f32, name="stats_maxe")
            stats_sumabs = ppool.tile([128, PAIRS], f32, name="stats_sumabs")
            stats_d = ppool.tile([128, PAIRS], f32, name="stats_d")
            stats_rd = ppool.tile([128, PAIRS], f32, name="stats_rd")
            stats_sc = ppool.tile([128, PAIRS], f32, name="stats_sc")

            # vsum accumulator: [64 c, 256 q], accumulated over all pairs
            vsum_ps = vsum_pool.tile([A, Q], f32, name="vsum_ps")

            # ---- prologue: transposes + projections ----
            with tc.tile_pool(name="tps", bufs=4, space="PSUM") as tps_pool:
                for src, dst in ((kt_sb, ktT_sb), (qt_sb, qtT_sb)):
                    for c in range(2):       # k-chunk
                        for tc2 in range(2):  # t-chunk
                            ps = tps_pool.tile([128, 128], f32, tag="tps")
                            nc.tensor.transpose(
                                ps[:], src[c][:, tc2 * 128:(tc2 + 1) * 128],
                                ident_sb[:])
                            nc.scalar.copy(
                                dst[tc2][:, c * 128:(c + 1) * 128], ps[:])

            def emit_projection(w_sb, dst, pool, tag="proj"):
                ps = pool.tile([128, PAIRS], f32, tag=tag)
                for par in range(2):
                    for tc2 in range(2):
                        rhs = ktT_sb[tc2][:].rearrange(
                            "p (j r) -> p r j", r=2)[:, par:par + 1, :]
                        nc.tensor.matmul(
                            ps[par * 64:(par + 1) * 64, :],
                            w_sb[:, tc2 * 64:(tc2 + 1) * 64],
                            rhs,
                            start=(tc2 == 0), stop=(tc2 == 1))
                nc.scalar.copy(dst[:], ps[:])

            with tc.tile_pool(name="proj", bufs=2, space="PSUM") as proj_pool:
                emit_projection(wkt_sb, key2_sb, proj_pool)
                # qT2: query projection duplicated on both partition halves
                ps = proj_pool.tile([128, Q], f32, tag="proj")
                for par in range(2):
                    for tc2 in range(2):
                        nc.tensor.matmul(
                            ps[par * 64:(par + 1) * 64, :],
                            wqt_sb[:, tc2 * 64:(tc2 + 1) * 64],
                            qtT_sb[tc2][:],
                            start=(tc2 == 0), stop=(tc2 == 1))
                nc.scalar.copy(qT2_sb[:], ps[:])

            # ---- main loop (software-pipelined emission) ----
            # Engines are in-order queues, so emission order defines each
            # engine's schedule.  Per iteration `it` (one group of G pairs):
            #   builds for a future supergroup | mms+exp(it) | mul(it-1) |
            #   when a supergroup s completes its muls: maxe(s), abs(s),
            #   stats(s), W(s), SEL matmuls(s)
            abs_engines = [x.strip() for x in abs_split.split(",") if x.strip()]
            nsg_total = NG // SGG
            SGQ = SGP * Q  # free size of a supergroup e/n tile
            with (
                tc.tile_pool(name="lhsT", bufs=4) as lhsT_pool,
                tc.tile_pool(name="wsel", bufs=3) as w_pool,
                tc.tile_pool(name="ebuf", bufs=6) as e_pool,
                tc.tile_pool(name="nbuf", bufs=10) as n_pool,
                tc.tile_pool(name="scr_ps", bufs=1, space="PSUM") as scr_ps_pool,
                tc.tile_pool(name="logits_ps", bufs=3, space="PSUM") as lg_pool,
            ):
                lhsT_tiles = {}
                lg_tiles = {}
                e_tiles = {}
                n_tiles = {}

                def bcast_cols(ap, n):
                    # [128, m] AP -> [128, m, n] with a stride-0 inner dim
                    return bass.AP(ap.tensor, ap.offset,
                                   list(ap.ap) + [[0, n]])

                def stage_build_sg(sg):
                    # one batched gpsimd op builds all SGP lhsT tiles:
                    # lhsT[p, jj*C + c] = wlgx[p, jj*C + c] * key2[p, j0+jj]
                    lhsT_sg = lhsT_pool.tile([128, SGP * lhsT_cols], fr,
                                             tag="lhsT")
                    build_eng.tensor_tensor(
                        lhsT_sg[:],
                        wlgx_sb[:],
                        bcast_cols(key2_sb[:, sg * SGP:(sg + 1) * SGP],
                                   lhsT_cols),
                        OP.mult)
                    for jj in range(SGP):
                        lhsT_tiles[sg * SGP + jj] = lhsT_sg[
                            :, jj * lhsT_cols:(jj + 1) * lhsT_cols]

                def stage_mm_exp(g):
                    lg = lg_pool.tile([128, G * Q], f32, tag="lg")
                    lg_tiles[g] = lg
                    for jj in range(G):
                        j = g * G + jj
                        lhsT = lhsT_tiles.pop(j)
                        if lhsT_cols == 128:
                            nc.tensor.matmul(
                                lg[:, jj * Q:(jj + 1) * Q],
                                lhsT, qT2_sb[:],
                                start=True, stop=True)
                        else:
                            for par in range(2):
                                nc.tensor.matmul(
                                    lg[par * 64:(par + 1) * 64,
                                       jj * Q:(jj + 1) * Q],
                                    lhsT[par * 64:(par + 1) * 64, :],
                                    qT2_sb[par * 64:(par + 1) * 64, :],
                                    start=True, stop=True)
                    e = e_pool.tile([128, G * Q], f32, tag="e", name="e_g")
                    nc.scalar.activation(e[:], lg[:], AF.Exp)
                    e_tiles[g] = e

                def stage_mul_maxe(g):
                    lg = lg_tiles.pop(g)
                    e = e_tiles.pop(g)
                    n = n_pool.tile([128, G * Q], fr, tag="n", name="n_g")
                    nc.vector.tensor_tensor(n[:], lg[:], e[:], OP.mult)
                    n_tiles[g] = n
                    nc.vector.reduce_max(
                        stats_maxe[:, g * G:(g + 1) * G],
                        e[:].rearrange("p (j q) -> p j q", q=Q),
                        axis=AX.X)

                def stage_abs(g):
                    n = n_tiles[g]
                    if abs_engines[g % len(abs_engines)] == "dve":
                        nc.vector.reduce_sum(
                            stats_sumabs[:, g * G:(g + 1) * G],
                            as_f32(n[:]).rearrange("p (j q) -> p j q", q=Q),
                            axis=AX.X, apply_absolute_value=True)
                        return
                    for jj in range(G):
                        j = g * G + jj
                        scr = scr_ps_pool.tile([128, Q], f32, tag="scr")
                        nc.scalar.activation(
                            scr[:], as_f32(n[:, jj * Q:(jj + 1) * Q]), AF.Abs,
                            accum_out=stats_sumabs[:, j:j + 1])

                def stage_sel(s):
                    j0 = s * SGP
                    sl = slice(j0, j0 + SGP)
                    stats_eng = nc.gpsimd if stats_engine == "gps" else nc.vector
                    stats_eng.tensor_tensor(
                        stats_d[:, sl], stats_sumabs[:, sl],
                        stats_maxe[:, sl], OP.add)
                    nc.vector.reciprocal(stats_rd[:, sl], stats_d[:, sl])
                    stats_eng.tensor_tensor(
                        stats_sc[:, sl], val2_sb[:, sl],
                        stats_rd[:, sl], OP.mult)
                    w_sg = w_pool.tile([128, SGP * A], fr, tag="w")
                    build_eng.tensor_tensor(
                        w_sg[:], sel2x_sb[:],
                        bcast_cols(stats_sc[:, sl], A), OP.mult)
                    for j2 in range(j0, j0 + SGP):
                        jj = j2 - j0
                        g2, jj2 = j2 // G, j2 % G
                        nt = n_tiles[g2]
                        nc.tensor.matmul(
                            vsum_ps[:],
                            w_sg[:, jj * A:(jj + 1) * A],
                            nt[:, jj2 * Q:(jj2 + 1) * Q],
                            start=(j2 == 0), stop=(j2 == PAIRS - 1),
                            skip_group_check=True)

                nsg_pf = max(1, (pf + 1) // SGG)
                for sgb in range(min(nsg_pf, nsg_total)):
                    stage_build_sg(sgb)
                sel_done = 0
                for it in range(NG + 3):
                    if it == 1:
                        # value projection deferred off the critical path
                        emit_projection(wvdt_sb, val2_sb, lg_pool, tag="lg")
                    if it % SGG == 0:
                        sgb = it // SGG + nsg_pf
                        if sgb < nsg_total:
                            stage_build_sg(sgb)
                    if it < NG:
                        stage_mm_exp(it)
                    if 0 <= it - 1 < NG:
                        stage_mul_maxe(it - 1)
                    if 0 <= it - 2 < NG:
                        stage_abs(it - 2)
                        while (sel_done < nsg_total
                               and it - 2 >= sel_done * SGG + SGG - 1
                               + sel_lag):
                            stage_sel(sel_done)
                            sel_done += 1
                while sel_done < nsg_total:
                    stage_sel(sel_done)
                    sel_done += 1

            # ---- epilogue ----
            with (
                tc.tile_pool(name="epi", bufs=2) as epi_pool,
                tc.tile_pool(name="epi_ps", bufs=2, space="PSUM") as epi_ps,
            ):
                vs_sb = epi_pool.tile([A, Q], fr, name="vs_sb")
                nc.scalar.copy(vs_sb[:], vsum_ps[:])
                for h in range(2):
                    ops = epi_ps.tile([128, T], f32, tag="ops")
                    nc.tensor.matmul(
                        ops[:],
                        vs_sb[:, h * 128:(h + 1) * 128],
                        wvut_r[:],
                        start=True, stop=True)
                    osb = epi_pool.tile([128, T], f32, tag="osb")
                    nc.scalar.copy(osb[:], ops[:])
                    nc.sync.dma_start(out[h * 128:(h + 1) * 128, :], osb[:])

    nc.compile()
    return nc


def get_nc():
    key = (BUILD_ENGINE, MM_DTYPE, ABS_SPLIT, SEL_LAG, MAXE_MODE, PF,
           STATS_ENGINE)
    if key not in _nc_cache:
        _nc_cache[key] = build_program(*key)
    return _nc_cache[key]


def make_in_maps(key_tokens, query_tokens, Wk, Wq, Wa, Wvd, Wvu):
    """Host-side sharding + weight layout packing (all small/cheap)."""
    f = np.float32

    def pack_T(w):  # [A, T] -> [128, 128]: chunked transpose
        return np.ascontiguousarray(
            np.concatenate([w[:, :128].T, w[:, 128:].T], axis=1), dtype=f)

    wkt = pack_T(np.asarray(Wk, f))
    wqt = pack_T(np.asarray(Wq, f))
    wvdt = pack_T(np.asarray(Wvd, f))
    wa = np.asarray(Wa, f)
    wat2 = np.ascontiguousarray(np.concatenate([wa.T, wa.T], axis=0))
    wabd = np.zeros((128, 128), f)
    wabd[:64, :64] = wa.T
    wabd[64:, 64:] = wa.T
    wvut = np.ascontiguousarray(np.asarray(Wvu, f).T)  # [64, 256]
    sel2 = np.concatenate([np.eye(A, dtype=f), np.eye(A, dtype=f)], axis=0)
    sel2x = np.ascontiguousarray(np.tile(sel2, (1, 8)))
    wlg = wabd if MM_DTYPE == "f32r" else wat2
    wlgx = np.zeros((128, 8 * 128), f)
    wlgx[:, :8 * wlg.shape[1]] = np.tile(wlg, (1, 8))
    ident = np.eye(128, dtype=f)

    in_maps = []
    for b in range(B):
        in_maps.append({
            "kt": np.ascontiguousarray(key_tokens[b], f),
            "qt": np.ascontiguousarray(query_tokens[b], f),
            "wkt": wkt, "wqt": wqt, "wvdt": wvdt, "wat2": wat2,
            "wabd": wabd, "wlgx": wlgx, "sel2x": sel2x,
            "wvut": wvut, "sel2": sel2, "ident": ident,
        })
    return in_maps


def kernel(key_tokens, query_tokens, Wk, Wq, Wa, Wvd, Wvu, _trace=False):
    from concourse.bass_utils import run_bass_kernel_spmd

    nc = get_nc()
    in_maps = make_in_maps(key_tokens, query_tokens, Wk, Wq, Wa, Wvd, Wvu)
    kwargs = {}
    if _trace:
        kwargs = dict(trace=True, stitch_traces=False)
    res = run_bass_kernel_spmd(nc, in_maps, core_ids=list(range(B)), **kwargs)
    out = np.stack([np.asarray(res.results[i]["out"]) for i in range(B)], axis=0)
    if _trace:
        return out, res
    return out


if __name__ == "__main__":
    rng = np.random.default_rng(0)
    demo = {
        "key_tokens": rng.standard_normal((B, K, T), dtype=np.float32),
        "query_tokens": rng.standard_normal((B, Q, T), dtype=np.float32),
        "Wk": rng.standard_normal((A, T), dtype=np.float32) * 0.06,
        "Wq": rng.standard_normal((A, T), dtype=np.float32) * 0.06,
        "Wa": rng.standard_normal((A, A), dtype=np.float32) * 0.12,
        "Wvd": rng.standard_normal((A, T), dtype=np.float32) * 0.06,
        "Wvu": rng.standard_normal((T, A), dtype=np.float32) * 0.12,
    }
    o = kernel(**demo)
    print("kernel output", o.shape, o.dtype, float(np.abs(o).max()))

